# revision 2
# baseline (speedup 1.0000x reference)
"""2-layer GAT (PyG GATConv semantics) on 8 Trainium2 NeuronCores via Bass/Tile.

Strategy: node (dst) partitioning with degree bucketing.
 - Host: add self loops, sort nodes by in-degree, deal round-robin into
   8 cores x NT tiles x 128 partitions so each tile holds similar-degree
   nodes.  Per (node, edge-slot) the source node's table row is precomputed;
   pad slots point at a sentinel row whose a_src is -1e30 (=> exp -> 0).
 - Device phase 0: xw = x @ W1 per tile + attention dot products -> local
   feature table [NSH, 132]; AllGather -> global table.
 - Phase 1: per tile, W_t indirect row gathers (edge e of node d lands on
   partition d), segment softmax + weighted sum via free-axis DVE reductions,
   relu -> h; then h^T @ W2 and layer-2 attention dots -> small local table;
   AllGather.
 - Phase 2: same pattern at width 9 -> log_softmax -> output.
"""
import sys

sys.path.insert(0, "/opt/trn_rl_repo")

import numpy as np

import concourse.bass as bass
import concourse.bacc as bacc
import concourse.mybir as mybir
import concourse.tile as tile
from concourse.bass_utils import run_bass_kernel_spmd

P = 128
NCORES = 8
F_IN = 128
HEADS = 4
HID = 32
NCLS = 8
NEG_SLOPE = 0.2
EPS = 1e-20
NEG_BIG = -1e30
W1COLS = HEADS * HID          # 128
EXTW = W1COLS + HEADS         # 132 : [xw | a_src]
H2W = NCLS + 1                # 9   : [xw2 | a_src2]

f32 = mybir.dt.float32
i32 = mybir.dt.int32


# ----------------------------------------------------------------------------
# host-side graph preprocessing
# ----------------------------------------------------------------------------

def preprocess(N, edge_index, n_tiles_per_core):
    """Returns slot layout + per-core index arrays.

    slot id = core*NT*P + t*P + d  (table row within the allgathered table)
    """
    NT = n_tiles_per_core
    slots_pc = NT * P
    total_slots = slots_pc * NCORES
    npad_pc = (total_slots - N) // NCORES
    assert npad_pc * NCORES == total_slots - N and npad_pc >= 1

    src = np.concatenate([edge_index[0], np.arange(N, dtype=np.int64)]).astype(np.int64)
    dst = np.concatenate([edge_index[1], np.arange(N, dtype=np.int64)]).astype(np.int64)

    deg = np.bincount(dst, minlength=N)  # in-degree incl self loop
    order = np.argsort(deg, kind="stable")  # ascending

    # slot assignment: group t=0 holds npad_pc pads per core (partitions
    # 0..npad_pc-1), then reals dealt round robin over cores.
    cap0 = (P - npad_pc) * NCORES
    r = np.arange(N)
    core = np.empty(N, np.int64)
    t_of = np.empty(N, np.int64)
    d_of = np.empty(N, np.int64)
    m0 = r < cap0
    core[m0] = r[m0] % NCORES
    t_of[m0] = 0
    d_of[m0] = npad_pc + r[m0] // NCORES
    r2 = r[~m0] - cap0
    core[~m0] = r2 % NCORES
    t_of[~m0] = 1 + r2 // (P * NCORES)
    d_of[~m0] = (r2 % (P * NCORES)) // NCORES
    assert t_of.max() < NT

    # table row of node (node id -> row), in sorted-rank order
    table_row_sorted = core * slots_pc + t_of * P + d_of
    table_row = np.empty(N, np.int64)
    table_row[order] = table_row_sorted

    # per-group max degree = program tile width
    W_prog = np.zeros(NT, np.int64)
    np.maximum.at(W_prog, t_of, deg[order])
    W_prog = np.maximum(W_prog, 1)
    O_t = np.concatenate([[0], np.cumsum(W_prog)])
    SW = int(O_t[-1])

    # edge slot fill: for each edge, k = index among dst's edges
    dslot = table_row[dst]                     # slot id of dst
    eorder = np.argsort(dslot, kind="stable")
    ds_sorted = dslot[eorder]
    src_sorted = src[eorder]
    starts = np.searchsorted(ds_sorted, np.arange(total_slots))
    k_idx = np.arange(len(ds_sorted)) - starts[ds_sorted]

    e_core = ds_sorted // slots_pc
    e_rem = ds_sorted % slots_pc
    e_t = e_rem // P
    e_d = e_rem % P
    e_col = O_t[e_t] + k_idx
    assert (k_idx < W_prog[e_t]).all()

    src_all = np.zeros((NCORES, P, SW), np.int32)  # sentinel row 0 (pad row)
    src_all[e_core, e_d, e_col] = table_row[src_sorted].astype(np.int32)

    # node id living at each slot (-1 for pads)
    node_of_slot = np.full(total_slots, -1, np.int64)
    node_of_slot[table_row] = np.arange(N)

    return dict(
        NT=NT, SW=SW, W_prog=W_prog.astype(int).tolist(),
        O_t=O_t.astype(int).tolist(), npad_pc=int(npad_pc),
        src_all=src_all, table_row=table_row,
        node_of_slot=node_of_slot, slots_pc=slots_pc,
    )


# ----------------------------------------------------------------------------
# device program
# ----------------------------------------------------------------------------

def build_program(NT, SW, W_prog, O_t, npad_pc):
    NSH = NT * P
    nc = bacc.Bacc("TRN2", target_bir_lowering=False, debug=False,
                   num_devices=NCORES)

    t_xT = nc.dram_tensor("xT", [P, NSH], f32, kind="ExternalInput")
    t_src = nc.dram_tensor("srcall", [P, SW], i32, kind="ExternalInput")
    t_W1 = nc.dram_tensor("W1", [F_IN, W1COLS], f32, kind="ExternalInput")
    t_a1s = nc.dram_tensor("a1s", [P, W1COLS], f32, kind="ExternalInput")
    t_a1d = nc.dram_tensor("a1d", [P, W1COLS], f32, kind="ExternalInput")
    t_b1 = nc.dram_tensor("b1b", [P, W1COLS], f32, kind="ExternalInput")
    t_W2 = nc.dram_tensor("W2p", [W1COLS, NCLS], f32, kind="ExternalInput")
    t_a2s = nc.dram_tensor("a2s", [P, NCLS], f32, kind="ExternalInput")
    t_a2d = nc.dram_tensor("a2d", [P, NCLS], f32, kind="ExternalInput")
    t_b2 = nc.dram_tensor("b2b", [P, NCLS], f32, kind="ExternalInput")
    t_id = nc.dram_tensor("ident", [P, P], f32, kind="ExternalInput")
    t_out = nc.dram_tensor("OUT", [NSH, NCLS], f32, kind="ExternalOutput")

    with tile.TileContext(nc) as tc:
        with (
            tc.tile_pool(name="const", bufs=1) as cp,
            tc.tile_pool(name="p0", bufs=3) as p0,
            tc.tile_pool(name="pg", bufs=2) as pg,
            tc.tile_pool(name="pm", bufs=2) as pm,
            tc.tile_pool(name="ps", bufs=2, space="PSUM") as psp,
            tc.tile_pool(name="dram", bufs=1, space="DRAM") as dram,
        ):
            xw_loc = dram.tile([NSH, EXTW], f32)
            adst_loc = dram.tile([NSH, HEADS], f32)
            h2_loc = dram.tile([NSH, H2W], f32)
            adst2_loc = dram.tile([NSH, 1], f32)
            xw_g = dram.tile([NSH * NCORES, EXTW], f32, addr_space="Shared")
            h2_g = dram.tile([NSH * NCORES, H2W], f32, addr_space="Shared")

            # ---- resident constants
            W1_s = cp.tile([F_IN, W1COLS], f32)
            a1s_s = cp.tile([P, W1COLS], f32)
            a1d_s = cp.tile([P, W1COLS], f32)
            b1_s = cp.tile([P, W1COLS], f32)
            W2_s = cp.tile([W1COLS, NCLS], f32)
            a2s_s = cp.tile([P, NCLS], f32)
            a2d_s = cp.tile([P, NCLS], f32)
            b2_s = cp.tile([P, NCLS], f32)
            id_s = cp.tile([P, P], f32)
            src_s = cp.tile([P, SW], i32)
            for sb_t, dr_t in [(W1_s, t_W1), (a1s_s, t_a1s), (a1d_s, t_a1d),
                               (b1_s, t_b1), (W2_s, t_W2), (a2s_s, t_a2s),
                               (a2d_s, t_a2d), (b2_s, t_b2), (id_s, t_id),
                               (src_s, t_src)]:
                nc.sync.dma_start(out=sb_t[:], in_=dr_t[:])

            # ---- phase 0: feature table
            for t in range(NT):
                xT_t = p0.tile([P, P], f32, tag="xT")
                nc.sync.dma_start(out=xT_t[:], in_=t_xT[:, t * P:(t + 1) * P])
                ps = psp.tile([P, W1COLS], f32, space="PSUM", tag="ps0")
                nc.tensor.matmul(out=ps[:], lhsT=xT_t[:], rhs=W1_s[:],
                                 start=True, stop=True)
                ext_t = p0.tile([P, EXTW], f32, tag="ext")
                nc.scalar.copy(ext_t[:, 0:W1COLS], ps[:])
                tmp = p0.tile([P, W1COLS], f32, tag="tmp0")
                nc.vector.tensor_mul(out=tmp[:], in0=ps[:], in1=a1s_s[:])
                nc.vector.tensor_reduce(
                    out=ext_t[:, W1COLS:EXTW],
                    in_=tmp[:].rearrange("p (h c) -> p h c", h=HEADS),
                    axis=mybir.AxisListType.X, op=mybir.AluOpType.add)
                tmp2 = p0.tile([P, W1COLS], f32, tag="tmp0b")
                nc.vector.tensor_mul(out=tmp2[:], in0=ps[:], in1=a1d_s[:])
                adst_t = p0.tile([P, HEADS], f32, tag="adst0")
                nc.vector.tensor_reduce(
                    out=adst_t[:],
                    in_=tmp2[:].rearrange("p (h c) -> p h c", h=HEADS),
                    axis=mybir.AxisListType.X, op=mybir.AluOpType.add)
                if t == 0:
                    nc.vector.memset(ext_t[0:npad_pc, W1COLS:EXTW], NEG_BIG)
                nc.sync.dma_start(out=xw_loc[t * P:(t + 1) * P, :], in_=ext_t[:])
                nc.sync.dma_start(out=adst_loc[t * P:(t + 1) * P, :], in_=adst_t[:])

            nc.gpsimd.collective_compute(
                "AllGather", mybir.AluOpType.bypass,
                replica_groups=[list(range(NCORES))],
                ins=[xw_loc[:].opt()], outs=[xw_g[:].opt()])

            # ---- phase 1: layer-1 message passing
            Wmax = max(W_prog)
            for t in range(NT):
                Wt = W_prog[t]
                g_all = pg.tile([P, Wmax, EXTW], f32, tag="g1")
                for k in range(Wt):
                    nc.gpsimd.indirect_dma_start(
                        out=g_all[:, k, :], out_offset=None, in_=xw_g[:],
                        in_offset=bass.IndirectOffsetOnAxis(
                            ap=src_s[:, O_t[t] + k:O_t[t] + k + 1], axis=0))
                adst_t = p0.tile([P, HEADS], f32, tag="adst1")
                nc.sync.dma_start(out=adst_t[:],
                                  in_=adst_loc[t * P:(t + 1) * P, :])
                e_t = pm.tile([P, Wt, HEADS], f32, tag="e1")
                nc.vector.tensor_tensor(
                    out=e_t[:],
                    in0=g_all[:, 0:Wt, W1COLS:EXTW],
                    in1=adst_t[:].rearrange("p (w h) -> p w h", w=1)
                        .to_broadcast([P, Wt, HEADS]),
                    op=mybir.AluOpType.add)
                f_t = pm.tile([P, Wt, HEADS], f32, tag="f1")
                nc.scalar.activation(f_t[:], e_t[:],
                                     mybir.ActivationFunctionType.Prelu,
                                     alpha=NEG_SLOPE)
                ee_t = pm.tile([P, Wt, HEADS], f32, tag="ee1")
                nc.scalar.activation(ee_t[:], f_t[:],
                                     mybir.ActivationFunctionType.Exp)
                den_t = pm.tile([P, HEADS], f32, tag="den1")
                nc.vector.tensor_reduce(
                    out=den_t[:],
                    in_=ee_t[:].rearrange("p w h -> p h w"),
                    axis=mybir.AxisListType.X, op=mybir.AluOpType.add)
                nc.vector.tensor_scalar_add(den_t[:], den_t[:], EPS)
                rec_t = pm.tile([P, HEADS], f32, tag="rec1")
                nc.vector.reciprocal(rec_t[:], den_t[:])
                al_t = pm.tile([P, Wt, HEADS], f32, tag="al1")
                nc.vector.tensor_tensor(
                    out=al_t[:], in0=ee_t[:],
                    in1=rec_t[:].rearrange("p (w h) -> p w h", w=1)
                        .to_broadcast([P, Wt, HEADS]),
                    op=mybir.AluOpType.mult)
                tm_t = pm.tile([P, Wt, W1COLS], f32, tag="tm1")
                nc.vector.tensor_tensor(
                    out=tm_t[:].rearrange("p w (h c) -> p w h c", h=HEADS),
                    in0=g_all[:, 0:Wt, 0:W1COLS]
                        .rearrange("p w (h c) -> p w h c", h=HEADS),
                    in1=al_t[:].rearrange("p w (h c) -> p w h c", c=1)
                        .to_broadcast([P, Wt, HEADS, HID]),
                    op=mybir.AluOpType.mult)
                h_t = pm.tile([P, W1COLS], f32, tag="h1")
                nc.vector.tensor_reduce(
                    out=h_t[:],
                    in_=tm_t[:].rearrange("p w f -> p f w"),
                    axis=mybir.AxisListType.X, op=mybir.AluOpType.add)
                nc.vector.tensor_add(out=h_t[:], in0=h_t[:], in1=b1_s[:])
                hr_t = pm.tile([P, W1COLS], f32, tag="hr1")
                nc.scalar.activation(hr_t[:], h_t[:],
                                     mybir.ActivationFunctionType.Relu)
                psT = psp.tile([P, P], f32, space="PSUM", tag="psT")
                nc.tensor.transpose(out=psT[:], in_=hr_t[:], identity=id_s[:])
                hT_t = pm.tile([P, P], f32, tag="hT")
                nc.scalar.copy(hT_t[:], psT[:])
                ps2 = psp.tile([P, NCLS], f32, space="PSUM", tag="ps2")
                nc.tensor.matmul(out=ps2[:], lhsT=hT_t[:], rhs=W2_s[:],
                                 start=True, stop=True)
                h2e_t = pm.tile([P, H2W], f32, tag="h2e")
                nc.scalar.copy(h2e_t[:, 0:NCLS], ps2[:])
                t8 = pm.tile([P, NCLS], f32, tag="t8")
                nc.vector.tensor_mul(out=t8[:], in0=ps2[:], in1=a2s_s[:])
                nc.vector.tensor_reduce(
                    out=h2e_t[:, NCLS:H2W], in_=t8[:],
                    axis=mybir.AxisListType.X, op=mybir.AluOpType.add)
                t8b = pm.tile([P, NCLS], f32, tag="t8b")
                nc.vector.tensor_mul(out=t8b[:], in0=ps2[:], in1=a2d_s[:])
                adst2_t = pm.tile([P, 1], f32, tag="adst2w")
                nc.vector.tensor_reduce(
                    out=adst2_t[:], in_=t8b[:],
                    axis=mybir.AxisListType.X, op=mybir.AluOpType.add)
                if t == 0:
                    nc.vector.memset(h2e_t[0:npad_pc, NCLS:H2W], NEG_BIG)
                nc.sync.dma_start(out=h2_loc[t * P:(t + 1) * P, :], in_=h2e_t[:])
                nc.sync.dma_start(out=adst2_loc[t * P:(t + 1) * P, :],
                                  in_=adst2_t[:])

            nc.gpsimd.collective_compute(
                "AllGather", mybir.AluOpType.bypass,
                replica_groups=[list(range(NCORES))],
                ins=[h2_loc[:].opt()], outs=[h2_g[:].opt()])

            # ---- phase 2: layer-2 message passing + log_softmax
            for t in range(NT):
                Wt = W_prog[t]
                g2 = pg.tile([P, Wmax, H2W], f32, tag="g2")
                for k in range(Wt):
                    nc.gpsimd.indirect_dma_start(
                        out=g2[:, k, :], out_offset=None, in_=h2_g[:],
                        in_offset=bass.IndirectOffsetOnAxis(
                            ap=src_s[:, O_t[t] + k:O_t[t] + k + 1], axis=0))
                ad2_t = p0.tile([P, 1], f32, tag="ad2")
                nc.sync.dma_start(out=ad2_t[:],
                                  in_=adst2_loc[t * P:(t + 1) * P, :])
                e2_t = pm.tile([P, Wt], f32, tag="e2")
                nc.vector.tensor_tensor(
                    out=e2_t[:].rearrange("p (w o) -> p w o", o=1),
                    in0=g2[:, 0:Wt, NCLS:H2W],
                    in1=ad2_t[:].rearrange("p (w o) -> p w o", w=1)
                        .to_broadcast([P, Wt, 1]),
                    op=mybir.AluOpType.add)
                f2_t = pm.tile([P, Wt], f32, tag="f2")
                nc.scalar.activation(f2_t[:], e2_t[:],
                                     mybir.ActivationFunctionType.Prelu,
                                     alpha=NEG_SLOPE)
                ee2_t = pm.tile([P, Wt], f32, tag="ee2")
                nc.scalar.activation(ee2_t[:], f2_t[:],
                                     mybir.ActivationFunctionType.Exp)
                den2_t = pm.tile([P, 1], f32, tag="den2")
                nc.vector.tensor_reduce(out=den2_t[:], in_=ee2_t[:],
                                        axis=mybir.AxisListType.X,
                                        op=mybir.AluOpType.add)
                nc.vector.tensor_scalar_add(den2_t[:], den2_t[:], EPS)
                rec2_t = pm.tile([P, 1], f32, tag="rec2")
                nc.vector.reciprocal(rec2_t[:], den2_t[:])
                al2_t = pm.tile([P, Wt], f32, tag="al2")
                nc.vector.tensor_scalar_mul(al2_t[:], ee2_t[:], rec2_t[:])
                tm2_t = pm.tile([P, Wt, NCLS], f32, tag="tm2")
                nc.vector.tensor_tensor(
                    out=tm2_t[:], in0=g2[:, 0:Wt, 0:NCLS],
                    in1=al2_t[:].rearrange("p (w c) -> p w c", c=1)
                        .to_broadcast([P, Wt, NCLS]),
                    op=mybir.AluOpType.mult)
                o_t = pm.tile([P, NCLS], f32, tag="o2")
                nc.vector.tensor_reduce(
                    out=o_t[:], in_=tm2_t[:].rearrange("p w c -> p c w"),
                    axis=mybir.AxisListType.X, op=mybir.AluOpType.add)
                nc.vector.tensor_add(out=o_t[:], in0=o_t[:], in1=b2_s[:])
                # log_softmax
                mx_t = pm.tile([P, 1], f32, tag="mx")
                nc.vector.tensor_reduce(out=mx_t[:], in_=o_t[:],
                                        axis=mybir.AxisListType.X,
                                        op=mybir.AluOpType.max)
                om_t = pm.tile([P, NCLS], f32, tag="om")
                nc.vector.tensor_scalar(out=om_t[:], in0=o_t[:],
                                        scalar1=mx_t[:], scalar2=None,
                                        op0=mybir.AluOpType.subtract)
                ex_t = pm.tile([P, NCLS], f32, tag="ex")
                nc.scalar.activation(ex_t[:], om_t[:],
                                     mybir.ActivationFunctionType.Exp)
                s_t = pm.tile([P, 1], f32, tag="s2")
                nc.vector.tensor_reduce(out=s_t[:], in_=ex_t[:],
                                        axis=mybir.AxisListType.X,
                                        op=mybir.AluOpType.add)
                ls_t = pm.tile([P, 1], f32, tag="ls")
                nc.scalar.activation(ls_t[:], s_t[:],
                                     mybir.ActivationFunctionType.Ln)
                res_t = pm.tile([P, NCLS], f32, tag="res")
                nc.vector.tensor_scalar(out=res_t[:], in0=om_t[:],
                                        scalar1=ls_t[:], scalar2=None,
                                        op0=mybir.AluOpType.subtract)
                nc.sync.dma_start(out=t_out[t * P:(t + 1) * P, :], in_=res_t[:])

    nc.compile()
    return nc


# ----------------------------------------------------------------------------
# entry point
# ----------------------------------------------------------------------------

_CACHE = {}


def run_gat(x, edge_index, W1, att_src1, att_dst1, b1, W2, att_src2, att_dst2,
            b2, n_tiles_per_core):
    N = x.shape[0]
    pre = preprocess(N, np.asarray(edge_index, np.int64), n_tiles_per_core)
    NT, SW = pre["NT"], pre["SW"]
    slots_pc = pre["slots_pc"]

    key = (N, NT, SW, tuple(pre["W_prog"]))
    if key not in _CACHE:
        _CACHE[key] = build_program(NT, SW, pre["W_prog"], pre["O_t"],
                                    pre["npad_pc"])
    nc = _CACHE[key]

    # per-core xT: [P, slots_pc] with x rows of the core's slots (pad -> 0)
    node_of_slot = pre["node_of_slot"].reshape(NCORES, slots_pc)
    x_ext = np.vstack([np.asarray(x, np.float32),
                       np.zeros((1, F_IN), np.float32)])
    bcast = lambda v, w: np.tile(np.asarray(v, np.float32).reshape(1, w), (P, 1))
    common = {
        "W1": np.asarray(W1, np.float32),
        "a1s": bcast(att_src1, W1COLS),
        "a1d": bcast(att_dst1, W1COLS),
        "b1b": bcast(b1, W1COLS),
        "W2p": np.asarray(W2, np.float32),
        "a2s": bcast(att_src2, NCLS),
        "a2d": bcast(att_dst2, NCLS),
        "b2b": bcast(b2, NCLS),
        "ident": np.eye(P, dtype=np.float32),
    }
    in_maps = []
    for c in range(NCORES):
        xs = x_ext[node_of_slot[c]]            # [slots_pc, F_IN]
        in_maps.append({
            "xT": np.ascontiguousarray(xs.T),
            "srcall": pre["src_all"][c],
            **common,
        })

    res = run_bass_kernel_spmd(nc, in_maps, core_ids=list(range(NCORES)))

    out = np.empty((N, NCLS), np.float32)
    for c in range(NCORES):
        o = res.results[c]["OUT"]              # [slots_pc, NCLS]
        mask = node_of_slot[c] >= 0
        out[node_of_slot[c][mask]] = o[mask]
    return out


def kernel(x, edge_index, W1, att_src1, att_dst1, b1, W2, att_src2, att_dst2,
           b2):
    x = np.asarray(x)
    return run_gat(x, edge_index, W1, att_src1, att_dst1, b1, W2,
                   att_src2, att_dst2, b2,
                   n_tiles_per_core=(x.shape[0] + NCORES * P - 1)
                   // (NCORES * P) if x.shape[0] != 50000 else 49)


# revision 20
# speedup vs baseline: 120.5290x; 120.5290x over previous
"""2-layer GAT (PyG GATConv semantics) on 8 Trainium2 NeuronCores via Bass/Tile.

Strategy: node (dst) partitioning with degree bucketing.
 - Host: add self loops, sort nodes by in-degree, deal round-robin into
   8 cores x NT tiles x 128 partitions so each tile holds similar-degree
   nodes.  Per (node, edge-slot) the source node's table row is precomputed;
   pad slots point at a sentinel row whose a_src is -1e30 (=> exp -> 0).
 - Device phase 0: xw = x @ W1 per tile + attention dot products -> local
   feature table [NSH, 132]; AllGather -> global table.
 - Phase 1: per tile, W_t indirect row gathers (edge e of node d lands on
   partition d), segment softmax + weighted sum via free-axis DVE reductions,
   relu -> h; then h^T @ W2 and layer-2 attention dots -> small local table;
   AllGather.
 - Phase 2: same pattern at width 9 -> log_softmax -> output.
"""
import sys

sys.path.insert(0, "/opt/trn_rl_repo")

import numpy as np

import concourse.bass as bass
import concourse.bacc as bacc
import concourse.mybir as mybir
import concourse.tile as tile
from concourse.bass_utils import run_bass_kernel_spmd

P = 128
NCORES = 8
F_IN = 128
HEADS = 4
HID = 32
NCLS = 8
NEG_SLOPE = 0.2
EPS = 1e-20
NEG_BIG = -1e30
W1COLS = HEADS * HID          # 128
EXTW = W1COLS + HEADS         # 132 : [xw | a_src]
H2W = NCLS + 1                # 9   : [xw2 | a_src2]

f32 = mybir.dt.float32
i32 = mybir.dt.int32


# ----------------------------------------------------------------------------
# host-side graph preprocessing
# ----------------------------------------------------------------------------

def preprocess(N, edge_index, n_tiles_per_core):
    """Slot layout + per-core index arrays.

    slot id = core*NT*P + t*P + d  (table row within the allgathered table)
    """
    NT = n_tiles_per_core
    slots_pc = NT * P
    total_slots = slots_pc * NCORES
    npad_pc = (total_slots - N) // NCORES
    assert npad_pc * NCORES == total_slots - N and npad_pc >= 1

    src = np.concatenate([edge_index[0], np.arange(N, dtype=np.int64)]).astype(np.int64)
    dst = np.concatenate([edge_index[1], np.arange(N, dtype=np.int64)]).astype(np.int64)

    deg = np.bincount(dst, minlength=N)  # in-degree incl self loop
    order = np.argsort(deg, kind="stable")  # ascending

    cap0 = (P - npad_pc) * NCORES
    r = np.arange(N)
    core = np.empty(N, np.int64)
    t_of = np.empty(N, np.int64)
    d_of = np.empty(N, np.int64)
    m0 = r < cap0
    core[m0] = r[m0] % NCORES
    t_of[m0] = 0
    d_of[m0] = npad_pc + r[m0] // NCORES
    r2 = r[~m0] - cap0
    core[~m0] = r2 % NCORES
    t_of[~m0] = 1 + r2 // (P * NCORES)
    d_of[~m0] = (r2 % (P * NCORES)) // NCORES
    assert t_of.max() < NT

    table_row_sorted = core * slots_pc + t_of * P + d_of
    table_row = np.empty(N, np.int64)
    table_row[order] = table_row_sorted

    W_prog = np.zeros(NT, np.int64)
    np.maximum.at(W_prog, t_of, deg[order])
    W_prog = np.maximum(W_prog, 1)
    O_t = np.concatenate([[0], np.cumsum(W_prog)])
    SW = int(O_t[-1])

    dslot = table_row[dst]
    is_self = (src == dst)
    # self-loop gets slot k=0 of its node: sort by (dslot, not-self)
    eorder = np.argsort(dslot * 2 + (~is_self).astype(np.int64), kind="stable")
    ds_sorted = dslot[eorder]
    src_sorted = src[eorder]
    starts = np.searchsorted(ds_sorted, np.arange(total_slots))
    k_idx = np.arange(len(ds_sorted)) - starts[ds_sorted]

    e_core = ds_sorted // slots_pc
    e_rem = ds_sorted % slots_pc
    e_t = e_rem // P
    e_d = e_rem % P
    e_col = O_t[e_t] + k_idx
    assert (k_idx < W_prog[e_t]).all()

    src_all = np.zeros((NCORES, P, SW), np.int32)  # sentinel row 0 (pad row)
    src_all[e_core, e_d, e_col] = table_row[src_sorted].astype(np.int32)

    node_of_slot = np.full(total_slots, -1, np.int64)
    node_of_slot[table_row] = np.arange(N)

    return dict(
        NT=NT, SW=SW, W_prog=W_prog.astype(int).tolist(),
        O_t=O_t.astype(int).tolist(), npad_pc=int(npad_pc),
        src_all=src_all, table_row=table_row,
        node_of_slot=node_of_slot, slots_pc=slots_pc,
    )


# ----------------------------------------------------------------------------
# device program emitters (shared between the real program and benchmarks)
# ----------------------------------------------------------------------------

class Ctx:
    """Pools + resident constant tiles."""

    def __init__(self, nc, tc, SW):
        from contextlib import ExitStack
        self.nc = nc
        self._es = ExitStack()
        self.cp = self._es.enter_context(tc.tile_pool(name="const", bufs=1))
        self.p0 = self._es.enter_context(tc.tile_pool(name="p0", bufs=3))
        self.pg = self._es.enter_context(tc.tile_pool(name="pg", bufs=3))
        self.pm = self._es.enter_context(tc.tile_pool(name="pm", bufs=2))
        self.psp = self._es.enter_context(
            tc.tile_pool(name="ps", bufs=2, space="PSUM"))
        self.dram = self._es.enter_context(
            tc.tile_pool(name="dram", bufs=1, space="DRAM"))

        cp = self.cp
        self.W1_s = cp.tile([F_IN, W1COLS], f32)
        self.a1s_s = cp.tile([P, W1COLS], f32)
        self.a1d_s = cp.tile([P, W1COLS], f32)
        self.b1_s = cp.tile([P, W1COLS], f32)
        self.W2_s = cp.tile([W1COLS, NCLS], f32)
        self.a2s_s = cp.tile([P, NCLS], f32)
        self.a2d_s = cp.tile([P, NCLS], f32)
        self.b2_s = cp.tile([P, NCLS], f32)
        self.id_s = cp.tile([P, P], f32)
        self.src_s = cp.tile([P, SW], i32)

    def close(self):
        self._es.close()

    def load_consts(self, tens):
        nc = self.nc
        for sb_t, name in [(self.W1_s, "W1"), (self.a1s_s, "a1s"),
                           (self.a1d_s, "a1d"), (self.b1_s, "b1b"),
                           (self.W2_s, "W2p"), (self.a2s_s, "a2s"),
                           (self.a2d_s, "a2d"), (self.b2_s, "b2b"),
                           (self.id_s, "ident"), (self.src_s, "srcall")]:
            nc.sync.dma_start(out=sb_t[:], in_=tens[name][:])


def emit_phase0_tile(cx, t, t_xT, xw_loc, npad_pc, pad_tile=None):
    """pad_tile: None -> pads iff t==0 (sharded build); else bool."""
    if pad_tile is None:
        pad_tile = (t == 0)
    nc = cx.nc
    xT_t = cx.p0.tile([P, P], f32, tag="xT")
    nc.sync.dma_start(out=xT_t[:], in_=t_xT[:, t * P:(t + 1) * P])
    ps = cx.psp.tile([P, W1COLS], f32, space="PSUM", tag="ps0")
    nc.tensor.matmul(out=ps[:], lhsT=xT_t[:], rhs=cx.W1_s[:],
                     start=True, stop=True)
    ext_t = cx.p0.tile([P, EXTW], f32, tag="ext")
    nc.scalar.copy(ext_t[:, 0:W1COLS], ps[:])
    tmp = cx.p0.tile([P, W1COLS], f32, tag="tmp0")
    nc.vector.tensor_mul(out=tmp[:], in0=ps[:], in1=cx.a1s_s[:])
    nc.vector.tensor_reduce(
        out=ext_t[:, W1COLS:EXTW],
        in_=tmp[:].rearrange("p (h c) -> p h c", h=HEADS),
        axis=mybir.AxisListType.X, op=mybir.AluOpType.add)
    if pad_tile and npad_pc > 0:
        nc.vector.memset(ext_t[0:npad_pc, W1COLS:EXTW], NEG_BIG)
    nc.sync.dma_start(out=xw_loc[t * P:(t + 1) * P, :], in_=ext_t[:])


import os
N_SWQ = int(os.environ.get("N_SWQ", "1"))


def emit_gathers(cx, g_all, table_ap, t, Wt, O_t):
    nc = cx.nc
    for k in range(Wt):
        bi = nc.gpsimd.indirect_dma_start(
            out=g_all[:, k, :], out_offset=None, in_=table_ap,
            in_offset=bass.IndirectOffsetOnAxis(
                ap=cx.src_s[:, O_t[t] + k:O_t[t] + k + 1], axis=0))
        if N_SWQ > 1:
            q = k % N_SWQ
            bi.ins.queue = f"qPoolDynamic{q or ''}"


def emit_phase1_tile(cx, t, Wt, O_t, Wmax, xw_g, h2_loc,
                     npad_pc, gathers_only=False, sink=None, skip_sink=False):
    nc = cx.nc
    g_all = cx.pg.tile([P, Wmax, EXTW], xw_g.dtype, tag="g1")
    emit_gathers(cx, g_all, xw_g[:], t, Wt, O_t)
    if gathers_only:
        if skip_sink:
            return
        s_t = cx.pm.tile([P, HEADS], f32, tag="sink1")
        nc.vector.tensor_reduce(
            out=s_t[:],
            in_=g_all[:, 0:Wt, W1COLS:EXTW].rearrange("p w h -> p h w"),
            axis=mybir.AxisListType.X, op=mybir.AluOpType.add)
        nc.sync.dma_start(out=sink[t * P:(t + 1) * P, 0:HEADS], in_=s_t[:])
        return
    tmpd = cx.pm.tile([P, W1COLS], f32, tag="tmpd1")
    nc.vector.tensor_mul(out=tmpd[:], in0=g_all[:, 0, 0:W1COLS],
                         in1=cx.a1d_s[:])
    adst_t = cx.pm.tile([P, HEADS], f32, tag="adst1")
    nc.vector.tensor_reduce(
        out=adst_t[:], in_=tmpd[:].rearrange("p (h c) -> p h c", h=HEADS),
        axis=mybir.AxisListType.X, op=mybir.AluOpType.add)
    e_t = cx.pm.tile([P, Wt, HEADS], f32, tag="e1")
    nc.vector.tensor_tensor(
        out=e_t[:],
        in0=g_all[:, 0:Wt, W1COLS:EXTW],
        in1=adst_t[:].rearrange("p (w h) -> p w h", w=1)
            .to_broadcast([P, Wt, HEADS]),
        op=mybir.AluOpType.add)
    f_t = cx.pm.tile([P, Wt, HEADS], f32, tag="f1")
    nc.scalar.activation(f_t[:], e_t[:], mybir.ActivationFunctionType.Prelu,
                         alpha=NEG_SLOPE)
    ee_t = cx.pm.tile([P, Wt, HEADS], f32, tag="ee1")
    nc.scalar.activation(ee_t[:], f_t[:], mybir.ActivationFunctionType.Exp)
    den_t = cx.pm.tile([P, HEADS], f32, tag="den1")
    nc.vector.tensor_reduce(
        out=den_t[:], in_=ee_t[:].rearrange("p w h -> p h w"),
        axis=mybir.AxisListType.X, op=mybir.AluOpType.add)
    nc.vector.tensor_scalar_add(den_t[:], den_t[:], EPS)
    rec_t = cx.pm.tile([P, HEADS], f32, tag="rec1")
    nc.vector.reciprocal(rec_t[:], den_t[:])
    al_t = cx.pm.tile([P, Wt, HEADS], f32, tag="al1")
    nc.vector.tensor_tensor(
        out=al_t[:], in0=ee_t[:],
        in1=rec_t[:].rearrange("p (w h) -> p w h", w=1)
            .to_broadcast([P, Wt, HEADS]),
        op=mybir.AluOpType.mult)
    tm_t = cx.pm.tile([P, Wt, W1COLS], f32, tag="tm1")
    nc.vector.tensor_tensor(
        out=tm_t[:].rearrange("p w (h c) -> p w h c", h=HEADS),
        in0=g_all[:, 0:Wt, 0:W1COLS].rearrange("p w (h c) -> p w h c", h=HEADS),
        in1=al_t[:].rearrange("p w (h c) -> p w h c", c=1)
            .to_broadcast([P, Wt, HEADS, HID]),
        op=mybir.AluOpType.mult)
    h_t = cx.pm.tile([P, W1COLS], f32, tag="h1")
    nc.vector.tensor_reduce(
        out=h_t[:], in_=tm_t[:].rearrange("p w f -> p f w"),
        axis=mybir.AxisListType.X, op=mybir.AluOpType.add)
    nc.vector.tensor_add(out=h_t[:], in0=h_t[:], in1=cx.b1_s[:])
    hr_t = cx.pm.tile([P, W1COLS], f32, tag="hr1")
    nc.scalar.activation(hr_t[:], h_t[:], mybir.ActivationFunctionType.Relu)
    psT = cx.psp.tile([P, P], f32, space="PSUM", tag="psT")
    nc.tensor.transpose(out=psT[:], in_=hr_t[:], identity=cx.id_s[:])
    hT_t = cx.pm.tile([P, P], f32, tag="hT")
    nc.scalar.copy(hT_t[:], psT[:])
    ps2 = cx.psp.tile([P, NCLS], f32, space="PSUM", tag="ps2")
    nc.tensor.matmul(out=ps2[:], lhsT=hT_t[:], rhs=cx.W2_s[:],
                     start=True, stop=True)
    h2e_t = cx.pm.tile([P, H2W], f32, tag="h2e")
    nc.scalar.copy(h2e_t[:, 0:NCLS], ps2[:])
    t8 = cx.pm.tile([P, NCLS], f32, tag="t8")
    nc.vector.tensor_mul(out=t8[:], in0=ps2[:], in1=cx.a2s_s[:])
    nc.vector.tensor_reduce(out=h2e_t[:, NCLS:H2W], in_=t8[:],
                            axis=mybir.AxisListType.X, op=mybir.AluOpType.add)
    if t == 0 and npad_pc > 0:
        nc.vector.memset(h2e_t[0:npad_pc, NCLS:H2W], NEG_BIG)
    nc.sync.dma_start(out=h2_loc[t * P:(t + 1) * P, :], in_=h2e_t[:])


def emit_phase2_tile(cx, t, Wt, O_t, Wmax, h2_g, t_out):
    nc = cx.nc
    g2 = cx.pg.tile([P, Wmax, H2W], f32, tag="g2")
    emit_gathers(cx, g2, h2_g[:], t, Wt, O_t)
    tmpd2 = cx.pm.tile([P, NCLS], f32, tag="tmpd2")
    nc.vector.tensor_mul(out=tmpd2[:], in0=g2[:, 0, 0:NCLS], in1=cx.a2d_s[:])
    ad2_t = cx.pm.tile([P, 1], f32, tag="ad2")
    nc.vector.tensor_reduce(out=ad2_t[:], in_=tmpd2[:],
                            axis=mybir.AxisListType.X, op=mybir.AluOpType.add)
    e2_t = cx.pm.tile([P, Wt], f32, tag="e2")
    nc.vector.tensor_tensor(
        out=e2_t[:].rearrange("p (w o) -> p w o", o=1),
        in0=g2[:, 0:Wt, NCLS:H2W],
        in1=ad2_t[:].rearrange("p (w o) -> p w o", w=1).to_broadcast([P, Wt, 1]),
        op=mybir.AluOpType.add)
    f2_t = cx.pm.tile([P, Wt], f32, tag="f2")
    nc.scalar.activation(f2_t[:], e2_t[:], mybir.ActivationFunctionType.Prelu,
                         alpha=NEG_SLOPE)
    ee2_t = cx.pm.tile([P, Wt], f32, tag="ee2")
    nc.scalar.activation(ee2_t[:], f2_t[:], mybir.ActivationFunctionType.Exp)
    den2_t = cx.pm.tile([P, 1], f32, tag="den2")
    nc.vector.tensor_reduce(out=den2_t[:], in_=ee2_t[:],
                            axis=mybir.AxisListType.X, op=mybir.AluOpType.add)
    nc.vector.tensor_scalar_add(den2_t[:], den2_t[:], EPS)
    rec2_t = cx.pm.tile([P, 1], f32, tag="rec2")
    nc.vector.reciprocal(rec2_t[:], den2_t[:])
    al2_t = cx.pm.tile([P, Wt], f32, tag="al2")
    nc.vector.tensor_scalar_mul(al2_t[:], ee2_t[:], rec2_t[:])
    tm2_t = cx.pm.tile([P, Wt, NCLS], f32, tag="tm2")
    nc.vector.tensor_tensor(
        out=tm2_t[:], in0=g2[:, 0:Wt, 0:NCLS],
        in1=al2_t[:].rearrange("p (w c) -> p w c", c=1)
            .to_broadcast([P, Wt, NCLS]),
        op=mybir.AluOpType.mult)
    o_t = cx.pm.tile([P, NCLS], f32, tag="o2")
    nc.vector.tensor_reduce(out=o_t[:], in_=tm2_t[:].rearrange("p w c -> p c w"),
                            axis=mybir.AxisListType.X, op=mybir.AluOpType.add)
    nc.vector.tensor_add(out=o_t[:], in0=o_t[:], in1=cx.b2_s[:])
    mx_t = cx.pm.tile([P, 1], f32, tag="mx")
    nc.vector.tensor_reduce(out=mx_t[:], in_=o_t[:],
                            axis=mybir.AxisListType.X, op=mybir.AluOpType.max)
    om_t = cx.pm.tile([P, NCLS], f32, tag="om")
    nc.vector.tensor_scalar(out=om_t[:], in0=o_t[:], scalar1=mx_t[:],
                            scalar2=None, op0=mybir.AluOpType.subtract)
    ex_t = cx.pm.tile([P, NCLS], f32, tag="ex")
    nc.scalar.activation(ex_t[:], om_t[:], mybir.ActivationFunctionType.Exp)
    s_t = cx.pm.tile([P, 1], f32, tag="s2")
    nc.vector.tensor_reduce(out=s_t[:], in_=ex_t[:],
                            axis=mybir.AxisListType.X, op=mybir.AluOpType.add)
    ls_t = cx.pm.tile([P, 1], f32, tag="ls")
    nc.scalar.activation(ls_t[:], s_t[:], mybir.ActivationFunctionType.Ln)
    res_t = cx.pm.tile([P, NCLS], f32, tag="res")
    nc.vector.tensor_scalar(out=res_t[:], in0=om_t[:], scalar1=ls_t[:],
                            scalar2=None, op0=mybir.AluOpType.subtract)
    nc.sync.dma_start(out=t_out[t * P:(t + 1) * P, :], in_=res_t[:])


def declare_inputs(nc, NSH, SW):
    tens = {}
    tens["xT"] = nc.dram_tensor("xT", [P, NSH], f32, kind="ExternalInput")
    tens["srcall"] = nc.dram_tensor("srcall", [P, SW], i32, kind="ExternalInput")
    tens["W1"] = nc.dram_tensor("W1", [F_IN, W1COLS], f32, kind="ExternalInput")
    tens["a1s"] = nc.dram_tensor("a1s", [P, W1COLS], f32, kind="ExternalInput")
    tens["a1d"] = nc.dram_tensor("a1d", [P, W1COLS], f32, kind="ExternalInput")
    tens["b1b"] = nc.dram_tensor("b1b", [P, W1COLS], f32, kind="ExternalInput")
    tens["W2p"] = nc.dram_tensor("W2p", [W1COLS, NCLS], f32, kind="ExternalInput")
    tens["a2s"] = nc.dram_tensor("a2s", [P, NCLS], f32, kind="ExternalInput")
    tens["a2d"] = nc.dram_tensor("a2d", [P, NCLS], f32, kind="ExternalInput")
    tens["b2b"] = nc.dram_tensor("b2b", [P, NCLS], f32, kind="ExternalInput")
    tens["ident"] = nc.dram_tensor("ident", [P, P], f32, kind="ExternalInput")
    return tens


def build_program(NT, SW, W_prog, O_t, npad_pc, repl_p0=False):
    NSH = NT * P
    nc = bacc.Bacc("TRN2", target_bir_lowering=False, debug=False,
                   num_devices=NCORES)
    nxt = NSH * NCORES if repl_p0 else NSH
    tens = declare_inputs(nc, nxt, SW)
    t_out = nc.dram_tensor("OUT", [NSH, NCLS], f32, kind="ExternalOutput")

    with tile.TileContext(nc) as tc:
        cx = Ctx(nc, tc, SW)
        dram = cx.dram
        h2_loc = dram.tile([NSH, H2W], f32)
        h2_g = dram.tile([NSH * NCORES, H2W], f32, addr_space="Shared")

        cx.load_consts(tens)
        if repl_p0:
            # every core builds the FULL table locally (xT input covers all
            # table rows in global order; pad tiles are each core shard's
            # tile 0, i.e. global tile index g with g % NT == 0).
            xw_g = dram.tile([nxt, EXTW], f32)
            for g in range(NT * NCORES):
                emit_phase0_tile(cx, g, tens["xT"], xw_g, npad_pc,
                                 pad_tile=(g % NT == 0))
        else:
            xw_loc = dram.tile([NSH, EXTW], f32)
            xw_g = dram.tile([NSH * NCORES, EXTW], f32, addr_space="Shared")
            for t in range(NT):
                emit_phase0_tile(cx, t, tens["xT"], xw_loc, npad_pc)
            nc.gpsimd.collective_compute(
                "AllGather", mybir.AluOpType.bypass,
                replica_groups=[list(range(NCORES))],
                ins=[xw_loc[:].opt()], outs=[xw_g[:].opt()])
        Wmax = max(W_prog)
        for t in range(NT):
            emit_phase1_tile(cx, t, W_prog[t], O_t, Wmax, xw_g, h2_loc,
                             npad_pc)
        nc.gpsimd.collective_compute(
            "AllGather", mybir.AluOpType.bypass,
            replica_groups=[list(range(NCORES))],
            ins=[h2_loc[:].opt()], outs=[h2_g[:].opt()])
        for t in range(NT):
            emit_phase2_tile(cx, t, W_prog[t], O_t, Wmax, h2_g, t_out)
        cx.close()

    nc.compile()
    return nc


# ----------------------------------------------------------------------------
# entry point
# ----------------------------------------------------------------------------

_CACHE = {}
REPL_P0 = True   # replicate phase 0 on all cores (no AllGather-1)


def make_in_maps(pre, x, W1, att_src1, att_dst1, b1, W2, att_src2, att_dst2,
                 b2):
    node_of_slot = pre["node_of_slot"].reshape(NCORES, pre["slots_pc"])
    x_ext = np.vstack([np.asarray(x, np.float32),
                       np.zeros((1, F_IN), np.float32)])
    bcast = lambda v, w: np.tile(np.asarray(v, np.float32).reshape(1, w), (P, 1))
    common = {
        "W1": np.asarray(W1, np.float32),
        "a1s": bcast(att_src1, W1COLS),
        "a1d": bcast(att_dst1, W1COLS),
        "b1b": bcast(b1, W1COLS),
        "W2p": np.asarray(W2, np.float32),
        "a2s": bcast(att_src2, NCLS),
        "a2d": bcast(att_dst2, NCLS),
        "b2b": bcast(b2, NCLS),
        "ident": np.eye(P, dtype=np.float32),
    }
    in_maps = []
    if REPL_P0:
        xs_full = np.ascontiguousarray(
            x_ext[pre["node_of_slot"]].T)         # [F_IN, total_slots]
    for c in range(NCORES):
        if REPL_P0:
            xT = xs_full
        else:
            xT = np.ascontiguousarray(x_ext[node_of_slot[c]].T)
        in_maps.append({
            "xT": xT,
            "srcall": pre["src_all"][c],
            **common,
        })
    return in_maps, node_of_slot


def run_gat(x, edge_index, W1, att_src1, att_dst1, b1, W2, att_src2, att_dst2,
            b2, n_tiles_per_core):
    N = x.shape[0]
    pre = preprocess(N, np.asarray(edge_index, np.int64), n_tiles_per_core)

    key = (N, pre["NT"], pre["SW"], tuple(pre["W_prog"]), REPL_P0)
    if key not in _CACHE:
        _CACHE[key] = build_program(pre["NT"], pre["SW"], pre["W_prog"],
                                    pre["O_t"], pre["npad_pc"],
                                    repl_p0=REPL_P0)
    nc = _CACHE[key]

    in_maps, node_of_slot = make_in_maps(pre, x, W1, att_src1, att_dst1, b1,
                                         W2, att_src2, att_dst2, b2)
    res = run_bass_kernel_spmd(nc, in_maps, core_ids=list(range(NCORES)))

    out = np.empty((N, NCLS), np.float32)
    for c in range(NCORES):
        o = res.results[c]["OUT"]
        mask = node_of_slot[c] >= 0
        out[node_of_slot[c][mask]] = o[mask]
    return out


def kernel(x, edge_index, W1, att_src1, att_dst1, b1, W2, att_src2, att_dst2,
           b2):
    x = np.asarray(x)
    NT = (x.shape[0] + NCORES * P - 1) // (NCORES * P)
    if (NCORES * P * NT - x.shape[0]) % NCORES or NCORES * P * NT == x.shape[0]:
        NT += 1
    return run_gat(x, edge_index, W1, att_src1, att_dst1, b1, W2,
                   att_src2, att_dst2, b2, n_tiles_per_core=NT)
